# revision 9
# baseline (speedup 1.0000x reference)
"""Trainium2 Bass kernel for nn_FEM_best (dual-attention fusion module).

Decomposition over 8 NeuronCores: core c = b*4 + r, b in {0,1}, r:
  r=0: role S (computes E_s), pixel half 0
  r=1: role S, half 1
  r=2: role Q (computes E_q), half 0
  r=3: role Q, half 1

Data plane: each core computes the stem for (its image, its pixel half) and
remote-DMA-broadcasts it to its 3 group peers, making every projection,
score, and PV computation fully local.  Synchronization uses two tiny
AllGather rendezvous (R1 validates stem sends; R4 validates halo + conv
partial sends) plus the small z-denominator AllGather.  The concat-conv runs
with zero halos; boundary rows are fixed by small correction matmuls using
remote-exchanged E edge rows (mask-zeroed on the sender so receivers apply
them uniformly).
"""
import sys, os
sys.path.insert(0, '/opt/trn_rl_repo')
import numpy as np

import concourse.bass as bass
import concourse.mybir as mybir
import concourse.bacc as bacc
import concourse.tile as tile
from concourse import bass_utils

F32 = mybir.dt.float32
F32R = mybir.dt.float32r
I32 = mybir.dt.int32
AF = mybir.ActivationFunctionType
ALU = mybir.AluOpType
BF16 = mybir.dt.bfloat16
F16 = mybir.dt.float16

B, CIN, H, W = 2, 256, 64, 64
N = H * W                 # 4096
C = 128                   # inter channels
CH = 64                   # C//2
CR = 8                    # gate bottleneck
HALF = N // 2             # 2048
PW = W + 2                # padded row width 66
NROW_IN = 34              # input rows per half (32 + 2 halo)
TC = 512                  # tile free size
NJ = HALF // TC           # 4 t-chunks per half
NU = N // 128             # 32 u-chunks (local-half-first order)
EXP_BIAS = -40.0
GROUPS4 = [[0, 1, 2, 3], [4, 5, 6, 7]]
POOL_ABS_U = set()   # gpsimd cannot read PSUM; abs stays on DVE

_cache = {}


def build_program():
    if 'nc' in _cache:
        return _cache['nc']
    nc = bacc.Bacc("TRN2", target_bir_lowering=False, debug=False, num_devices=8)

    def din(name, shape, dt=F32):
        return nc.dram_tensor(name, list(shape), dt, kind="ExternalInput").ap()

    def dout(name, shape, dt=F32):
        return nc.dram_tensor(name, list(shape), dt, kind="ExternalOutput").ap()

    i_img = din("img", [2, 128, NROW_IN * PW], F16)
    i_wstem = din("wstem", [2, 9, 128, 128], F16)
    i_bstem = din("bstem", [128, 1])
    i_wproj = din("wproj", [128, 256], F16)     # [w1_mine | w1_other | w2_mine | w2_other]
    i_bx1 = din("bx1", [128, 1])
    i_bx2 = din("bx2", [128, 1])
    i_wv = din("wv", [128, 128], F16)
    i_bvrow = din("bvrow", [1, 128])
    i_w1t = din("w1t", [128, CR])
    i_b1 = din("b1", [CR, 1])
    i_w2t = din("w2t", [CR, 128])
    i_b2 = din("b2", [128, 1])
    i_wcc = din("wcc", [9, 128, 128], F16)      # my-role cin half, identity co
    i_zmsk = din("zmsk", [128, 8])        # per-core granule masks

    o_e = dout("o_e", [128, HALF])
    o_cc = dout("o_cc", [128, HALF], F16)  # raw conv partial (host adds + relu)
    o_corr = dout("o_corr", [128, 128])    # boundary-row corrections for neighbor

    rsem = nc.alloc_semaphore("rsem")
    lsem = nc.alloc_semaphore("lsem")

    def bcast(dst_tile, src_ap, delta):
        rdests = [None] * 8
        rdests[delta] = (0, delta)
        nc.gpsimd.remote_dma_broadcast(dst_tile[:], src_ap, rsem, lsem,
                                       rdests=rdests)

    with tile.TileContext(nc) as tc:
        with tc.tile_pool(name="per", bufs=1) as per, \
             tc.tile_pool(name="dram", bufs=1, space="DRAM") as dram:
            # ---- persistent tiles ----
            stem32 = per.tile([128, HALF], F32)      # my stem (residual use)
            stemc = per.tile([128, HALF], F16)       # f16 copy = send payload
            rx_s1 = per.tile([128, HALF], F16)       # peer stems (landing)
            rx_s2 = per.tile([128, HALF], F16)
            rx_s3 = per.tile([128, HALF], F16)
            x1 = per.tile([128, N], F16)
            x2 = per.tile([128, HALF], F16)
            vpm = per.tile([128, N], BF16)           # v pixel-major blocks
            gvec = per.tile([128, 1], F32)
            zslot = per.tile([128, NU], BF16)
            etile = per.tile([128, HALF], F32)
            rbc = per.tile([128, HALF], F32)
            bneg = per.tile([128, 1], F32)
            nc.gpsimd.memset(bneg[:], EXP_BIAS)
            ones_u = per.tile([128, 1], BF16)
            nc.gpsimd.memset(ones_u[:], 1.0)
            ones1p = per.tile([1, 128], BF16)
            nc.gpsimd.memset(ones1p[:], 1.0)
            tokz1 = per.tile([128, 1], F32)
            wcc = per.tile([128, 9, 128], F16)
            zmsk = per.tile([128, 8], F32)

            EPW = NROW_IN * PW + 2
            epadS = per.tile([128, EPW], F16)
            zsrc = per.tile([128, PW], F16)

            bar1_in = dram.tile([128, 4], F32)
            bar1_out = dram.tile([512, 4], F32)
            ag3_in = dram.tile([128, NU], BF16)
            ag3_out = dram.tile([512, NU], BF16)
            dsc = dram.tile([16, 128], BF16)

            # ================= Phase A: stem + own projections =================
            with nc.named_scope("pA_stem"), \
                 tc.tile_pool(name="pha", bufs=1) as pha, \
                 tc.tile_pool(name="psA", bufs=2, space="PSUM") as psA:
                wstem = pha.tile([128, 2, 9, 128], F16)
                nc.sync.dma_start(wstem[:], i_wstem[:].rearrange("a t p c -> p a t c"))
                bstem = pha.tile([128, 1], F32)
                nc.sync.dma_start(bstem[:], i_bstem[:])
                wproj = pha.tile([128, 256], F16)
                nc.sync.dma_start(wproj[:], i_wproj[:])
                bx1 = pha.tile([128, 1], F32)
                nc.sync.dma_start(bx1[:], i_bx1[:])
                bx2 = pha.tile([128, 1], F32)
                nc.sync.dma_start(bx2[:], i_bx2[:])
                wv = pha.tile([128, 128], F16)
                nc.sync.dma_start(wv[:], i_wv[:])
                bvrow = pha.tile([1, 128], F32)
                nc.sync.dma_start(bvrow[:], i_bvrow[:])
                bvbc = pha.tile([128, 128], F32)
                nc.gpsimd.partition_broadcast(bvbc[:], bvrow[:])
                img = pha.tile([128, 2, NROW_IN * PW + 2], F16)
                isrc = i_img[:].rearrange("a p x -> p a x")
                for r0, r1 in ((0, 6), (6, 12), (12, 20), (20, 27), (27, 34)):
                    nc.sync.dma_start(img[:, :, r0 * PW:r1 * PW],
                                      isrc[:, :, r0 * PW:r1 * PW])
                # deferred loads (off the img critical path)
                nc.sync.dma_start(wcc[:], i_wcc[:].rearrange("t p c -> p t c"))
                nc.sync.dma_start(zmsk[:], i_zmsk[:])
                nc.vector.memset(zsrc[:], 0.0)
                nc.sync.dma_start(epadS[:, 0:PW], zsrc[:])
                nc.sync.dma_start(epadS[:, 33 * PW:34 * PW], zsrc[:])
                lcol = epadS[:, PW:33 * PW].rearrange("p (r w) -> p r w", w=PW)[:, :, 0:1]
                nc.sync.dma_start(lcol, zsrc[:, 0:32].unsqueeze(2))
                rcol = epadS[:, PW + 65:33 * PW + 65].rearrange("p (r w) -> p r w", w=PW)[:, :, 0:1]
                nc.sync.dma_start(rcol, zsrc[:, 0:32].unsqueeze(2))
                nc.sync.dma_start(epadS[:, 34 * PW:EPW], zsrc[:, 0:2])

                wup = pha.tile([128, TC], F32)
                nc.vector.memset(wup[:], 0.0)

                for pc in range(NJ):
                    sl = slice(pc * TC, (pc + 1) * TC)
                    ps = psA.tile([128, TC], F32)
                    if pc == 0:
                        for _ in range(2):
                            nc.tensor.matmul(ps[:], wup[:, 0:128], wup[:],
                                             start=True, stop=True)
                    first = True
                    for ch in range(2):
                        for t in range(9):
                            dy, dx = t // 3 - 1, t % 3 - 1
                            off = (8 * pc + dy + 1) * PW + (dx + 1)
                            rhs = img[:, ch, off:off + 8 * PW].rearrange(
                                "p (r w) -> p r w", r=8)[:, :, 0:64]
                            nc.tensor.matmul(ps[:], wstem[:, ch, t, :], rhs,
                                             start=first, stop=(ch == 1 and t == 8))
                            first = False
                    nc.scalar.activation(stemc[:, sl], ps[:], AF.Relu, bias=bstem[:])
                    nc.scalar.activation(stem32[:, sl], ps[:], AF.Relu, bias=bstem[:])

                # ship my stem to the 3 group peers; rendezvous R1 validates
                bcast(rx_s1, stemc[:], 1)
                bcast(rx_s2, stemc[:], 2)
                bcast(rx_s3, stemc[:], 3)
                nc.gpsimd.trigger_dma(count=None, signals_writable=[bar1_in[:]])
                with nc.named_scope("R1"):
                    nc.gpsimd.collective_compute(
                        "AllGather", ALU.bypass, replica_groups=GROUPS4,
                        ins=[bar1_in.opt()], outs=[bar1_out.opt()])

                # own-stem projections (overlap with R1):
                # x1 rows 0:64 cols 0:HALF ; x2 rows 0:64 ; v cols 0:HALF
                for pc in range(NJ):
                    sl = slice(pc * TC, (pc + 1) * TC)
                    p1 = psA.tile([128, TC], F32, name="pp", bufs=3)
                    nc.tensor.matmul(p1[0:64, :], wproj[:, 0:64], stemc[:, sl],
                                     start=True, stop=True)
                    nc.vector.tensor_scalar(x1[0:64, sl], p1[0:64, :], bx1[0:64, :],
                                            None, ALU.add)
                    p2 = psA.tile([128, TC], F32, name="pp", bufs=3)
                    nc.tensor.matmul(p2[0:64, :], wproj[:, 128:192], stemc[:, sl],
                                     start=True, stop=True)
                    nc.vector.tensor_scalar(x2[0:64, sl], p2[0:64, :], bx2[0:64, :],
                                            None, ALU.add)
                for uc in range(16):
                    usl = slice(uc * 128, (uc + 1) * 128)
                    psv = psA.tile([128, TC], F32, name="pp", bufs=3)
                    nc.tensor.matmul(psv[:, 0:128], stemc[:, usl], wv[:],
                                     start=True, stop=True)
                    nc.vector.tensor_tensor(vpm[:, usl], psv[:, 0:128], bvbc[:], ALU.add)

                # R1 token: zeroed, fold into the weights used on rx stems
                tok1 = pha.tile([128, 4], F32, name="tok1")
                nc.sync.dma_start(tok1[:], bar1_out[0:128, :])
                nc.vector.tensor_scalar(tokz1[:], tok1[:, 0:1], 0.0, None, ALU.mult)
                wproj2 = pha.tile([128, 256], F16, name="wproj2")
                nc.vector.tensor_scalar(wproj2[:], wproj[:], tokz1[:, 0:1],
                                        None, ALU.add)
                wv2 = pha.tile([128, 128], F16, name="wv2")
                nc.vector.tensor_scalar(wv2[:], wv[:], tokz1[:, 0:1], None, ALU.add)

                # peer-stem projections (gated by R1 through wproj2/wv2)
                # x2-hi first: u=0 scores need the full x2
                for pc in range(NJ):
                    sl = slice(pc * TC, (pc + 1) * TC)
                    p6 = psA.tile([128, TC], F32, name="pp", bufs=3)
                    nc.tensor.matmul(p6[0:64, :], wproj2[:, 192:256], rx_s2[:, sl],
                                     start=True, stop=True)
                    nc.vector.tensor_scalar(x2[64:128, sl], p6[0:64, :], bx2[64:128, :],
                                            None, ALU.add)
                for pc in range(NJ):
                    sl = slice(pc * TC, (pc + 1) * TC)
                    slo = slice(HALF + pc * TC, HALF + (pc + 1) * TC)
                    p4 = psA.tile([128, TC], F32, name="pp", bufs=3)
                    nc.tensor.matmul(p4[0:64, :], wproj2[:, 64:128], rx_s2[:, sl],
                                     start=True, stop=True)
                    nc.vector.tensor_scalar(x1[64:128, sl], p4[0:64, :], bx1[64:128, :],
                                            None, ALU.add)
                    p3 = psA.tile([128, TC], F32, name="pp", bufs=3)
                    nc.tensor.matmul(p3[0:64, :], wproj2[:, 0:64], rx_s1[:, sl],
                                     start=True, stop=True)
                    nc.vector.tensor_scalar(x1[0:64, slo], p3[0:64, :], bx1[0:64, :],
                                            None, ALU.add)
                    p5 = psA.tile([128, TC], F32, name="pp", bufs=3)
                    nc.tensor.matmul(p5[0:64, :], wproj2[:, 64:128], rx_s3[:, sl],
                                     start=True, stop=True)
                    nc.vector.tensor_scalar(x1[64:128, slo], p5[0:64, :], bx1[64:128, :],
                                            None, ALU.add)
                for uc in range(16):
                    usl = slice(uc * 128, (uc + 1) * 128)
                    uslo = slice(HALF + uc * 128, HALF + (uc + 1) * 128)
                    psv2 = psA.tile([128, TC], F32, name="pp", bufs=3)
                    nc.tensor.matmul(psv2[:, 0:128], rx_s1[:, usl], wv2[:],
                                     start=True, stop=True)
                    nc.vector.tensor_tensor(vpm[:, uslo], psv2[:, 0:128], bvbc[:], ALU.add)


            # ================= Phase C: attention =================
            with nc.named_scope("pC_attn"), tc.tile_pool(name="phc", bufs=3) as phc, \
                 tc.tile_pool(name="psS", bufs=3, space="PSUM") as psS, \
                 tc.tile_pool(name="psO", bufs=1, space="PSUM") as psO:
                pv_ps = []
                for j in range(NJ):
                    pv_ps.append(psO.tile([128, TC], F32, name=f"pvps{j}"))
                LAG = 5
                pts = {}

                def emit_pv(uu):
                    uslv = slice(uu * 128, uu * 128 + 128)
                    ptv = pts.pop(uu)
                    for j in range(NJ):
                        tsl = slice(j * TC, (j + 1) * TC)
                        nc.tensor.matmul(pv_ps[j][:], vpm[:, uslv], ptv[:, tsl],
                                         start=(uu == 0), stop=(uu == NU - 1))

                for step in range(NU + LAG):
                    if step < NU:
                        u = step
                        usl = slice(u * 128, u * 128 + 128)
                        pabs = phc.tile([128, HALF], F32, name="pabs", bufs=6)
                        for j2 in range(2):
                            t2 = slice(j2 * 2 * TC, (j2 * 2 + 2) * TC)
                            sps = psS.tile([128, 2 * TC], F32, name="sps", bufs=2)
                            for jj in range(2):
                                nc.tensor.matmul(
                                    sps[:, jj * TC:(jj + 1) * TC], x1[:, usl],
                                    x2[:, (j2 * 2 + jj) * TC:(j2 * 2 + jj + 1) * TC],
                                    start=True, stop=True)
                            if u in POOL_ABS_U:
                                nc.gpsimd.tensor_scalar(pabs[:, t2].bitcast(I32),
                                                        sps[:].bitcast(I32),
                                                        0x7FFFFFFF, None,
                                                        ALU.bitwise_and)
                            else:
                                nc.vector.tensor_scalar(pabs[:, t2].bitcast(I32),
                                                        sps[:].bitcast(I32),
                                                        0x7FFFFFFF, None,
                                                        ALU.bitwise_and)
                        pt = phc.tile([128, HALF], BF16, name="p_t", bufs=LAG + 2)
                        with nc.allow_low_precision(reason="z partial in bf16; D error ~0.4% ok"):
                            nc.scalar.activation(pt[:], pabs[:], AF.Exp, bias=bneg[:],
                                                 accum_out=zslot[:, u:u + 1])
                        pts[u] = pt
                    if step >= LAG:
                        emit_pv(step - LAG)

                # gate: sigmoid(w2 @ relu(w1 @ mean(v) + b1) + b2)
                w1t = phc.tile([128, CR], F32, name="w1t")
                nc.sync.dma_start(w1t[:], i_w1t[:])
                b1 = phc.tile([CR, 1], F32, name="b1")
                nc.sync.dma_start(b1[:], i_b1[:])
                w2t = phc.tile([CR, 128], F32, name="w2t")
                nc.sync.dma_start(w2t[:], i_w2t[:])
                b2 = phc.tile([128, 1], F32, name="b2")
                nc.sync.dma_start(b2[:], i_b2[:])
                psum_v = psS.tile([128, 2 * TC], F32, name="sps", bufs=2)
                for uc in range(NU):
                    nc.tensor.matmul(psum_v[:, 0:1], vpm[:, uc * 128:(uc + 1) * 128],
                                     ones_u[:], start=(uc == 0), stop=(uc == NU - 1))
                vsum = phc.tile([128, 1], F32, name="vsum")
                nc.scalar.copy(vsum[:], psum_v[:, 0:1])
                psh = psS.tile([128, 2 * TC], F32, name="sps", bufs=2)
                nc.tensor.matmul(psh[0:CR, 0:1], w1t[:], vsum[:], start=True, stop=True)
                hgate = phc.tile([CR, 1], F32, name="hgate")
                nc.scalar.activation(hgate[:], psh[0:CR, 0:1], AF.Relu, bias=b1[:])
                psg = psS.tile([128, 2 * TC], F32, name="sps", bufs=2)
                nc.tensor.matmul(psg[:, 0:1], w2t[:], hgate[:], start=True, stop=True)
                nc.scalar.activation(gvec[:], psg[:, 0:1], AF.Sigmoid, bias=b2[:])

                nc.sync.dma_start(ag3_in[:], zslot[:])
                with nc.named_scope("ag3"):
                    nc.gpsimd.collective_compute(
                        "AllGather", ALU.bypass, replica_groups=GROUPS4,
                        ins=[ag3_in.opt()], outs=[ag3_out.opt()])

                # D = sum of the two other-role z partials for my pixels:
                # load all 4 group sections statically, masked-accumulate
                with nc.named_scope("pD_efin"):
                    zt = phc.tile([128, 128], BF16, name="zt")
                    for s in range(4):
                        nc.sync.dma_start(zt[:, s * 32:(s + 1) * 32],
                                          ag3_out[128 * s:128 * (s + 1), :])
                    dmine = phc.tile([128, 16], F32, name="dmine")
                    nc.vector.tensor_scalar(dmine[:], zt[:, 0:16], zmsk[:, 0:1],
                                            None, ALU.mult)
                    for k in range(1, 8):
                        s, g = k // 2, k % 2
                        nc.vector.scalar_tensor_tensor(
                            dmine[:], zt[:, s * 32 + g * 16:s * 32 + g * 16 + 16],
                            zmsk[:, k:k + 1], dmine[:], ALU.mult, ALU.add)
                    # small PE warm bridge keyed on zt (runs during the D math)
                    wps = psS.tile([128, 2 * TC], F32, name="sps", bufs=2)
                    for _ in range(0):
                        nc.tensor.matmul(wps[0:16, 0:16], zt[:, 0:16], zt[:, 0:16],
                                         start=True, stop=True)
                    rrec = phc.tile([128, 16], BF16, name="rrec")
                    drow = phc.tile([1, HALF], BF16, name="drow")
                    dscT = dsc[:].rearrange("c p -> p c")
                    for rb in range(2):
                        csl = slice(rb * 8, rb * 8 + 8)
                        with nc.allow_low_precision(reason="1/D in bf16, ~0.4% ok"):
                            nc.vector.reciprocal(rrec[:, csl], dmine[:, csl])
                        nc.sync.dma_start(dscT[:, csl], rrec[:, csl])
                        nc.sync.dma_start(
                            drow[:, rb * 2 * TC:(rb + 1) * 2 * TC],
                            dsc[rb * 8:rb * 8 + 8, :].rearrange("c p -> (c p)").unsqueeze(0))
                        t = psS.tile([128, 2 * TC], F32, name="sps", bufs=2)
                        for hh in range(2):
                            nc.tensor.matmul(
                                t[:, hh * TC:(hh + 1) * TC], ones1p[:],
                                drow[:, (rb * 2 + hh) * TC:(rb * 2 + hh + 1) * TC],
                                start=True, stop=True)
                        nc.scalar.copy(rbc[:, rb * 2 * TC:(rb + 1) * 2 * TC], t[:])

                    # E = (PV * g) * R + resid; ACT copies into padded conv input
                    for j in (0, 1, 2, 3):
                        tsl = slice(j * TC, (j + 1) * TC)
                        nc.vector.scalar_tensor_tensor(etile[:, tsl], pv_ps[j][:],
                                                       gvec[:], rbc[:, tsl],
                                                       ALU.mult, ALU.mult)
                        nc.gpsimd.tensor_tensor(etile[:, tsl], etile[:, tsl],
                                                stem32[:, tsl], ALU.add)
                        erows = epadS[:, PW + 1 + j * 8 * PW:PW + 1 + (j + 1) * 8 * PW]
                        nc.scalar.activation(
                            erows.rearrange("p (r w) -> p r w", w=PW)[:, :, 0:64],
                            etile[:, tsl].rearrange("p (r w) -> p r w", w=64),
                            AF.Identity)
                        nc.sync.dma_start(o_e[:, tsl], etile[:, tsl])


            # ============ Phase E: concat conv via per-role partials ============
            with nc.named_scope("pE_cc"), tc.tile_pool(name="phe", bufs=1) as phe, \
                 tc.tile_pool(name="psE", bufs=2, space="PSUM") as psE:
                ccout = phe.tile([128, HALF], F16, name="ccout")
                for pc in range(NJ):
                    ps = psE.tile([128, TC], F32, name="cps", bufs=2)
                    for t in range(9):
                        dy, dx = t // 3 - 1, t % 3 - 1
                        off = (8 * pc + dy + 1) * PW + (dx + 1)
                        rhs = epadS[:, off:off + 8 * PW].rearrange(
                            "p (r w) -> p r w", r=8)[:, :, 0:64]
                        nc.tensor.matmul(ps[:], wcc[:, t, :], rhs,
                                         start=(t == 0), stop=(t == 8))
                    nc.vector.tensor_copy(ccout[:, pc * TC:(pc + 1) * TC], ps[:])
                    nc.sync.dma_start(o_cc[:, pc * TC:(pc + 1) * TC],
                                      ccout[:, pc * TC:(pc + 1) * TC])

                # export boundary corrections for the other-half neighbor:
                # cols 0:64  = dy=-1 taps applied to my LAST E row
                # cols 64:128 = dy=+1 taps applied to my FIRST E row
                psT = psE.tile([128, 64], F32, name="psT", bufs=1)
                psB = psE.tile([128, 64], F32, name="psB", bufs=1)
                for dx in range(3):
                    nc.tensor.matmul(psT[:], wcc[:, dx, :],
                                     epadS[:, 32 * PW + dx:32 * PW + dx + 64],
                                     start=(dx == 0), stop=(dx == 2))
                    nc.tensor.matmul(psB[:], wcc[:, 6 + dx, :],
                                     epadS[:, PW + dx:PW + dx + 64],
                                     start=(dx == 0), stop=(dx == 2))
                corr = phe.tile([128, 128], F32, name="corr")
                nc.vector.tensor_copy(corr[:, 0:64], psT[:])
                nc.vector.tensor_copy(corr[:, 64:128], psB[:])
                nc.sync.dma_start(o_corr[:], corr[:])

    nc.compile()
    _cache['nc'] = nc
    return nc


# ====================== host-side preparation ======================

def _prep_inputs(inp):
    f32 = np.float32
    g = {k: np.asarray(v, f32) for k, v in inp.items()}
    eps = 1e-5

    def fold_stem(w, b, gam, be, m, v):
        s = gam / np.sqrt(v + eps)
        w_eff = w * s[:, None, None, None]
        b_eff = (b - m) * s + be
        wt = np.zeros((2, 9, 128, 128), f32)
        for ch in range(2):
            for t in range(9):
                wt[ch, t] = w_eff[:, ch * 128:(ch + 1) * 128, t // 3, t % 3].T
        return wt, b_eff.astype(f32).reshape(128, 1)

    ws_s, bs_s = fold_stem(g['ts_w'], g['ts_b'], g['ts_g'], g['ts_be'], g['ts_m'], g['ts_v'])
    ws_q, bs_q = fold_stem(g['tq_w'], g['tq_b'], g['tq_g'], g['tq_be'], g['tq_m'], g['tq_v'])

    s_cc = g['cc_g'] / np.sqrt(g['cc_v'] + eps)
    wcc_eff = g['cc_w'] * s_cc[:, None, None, None]     # [128, 256, 3, 3]
    bcc_eff = (g['cc_be'] - g['cc_m'] * s_cc).astype(f32).reshape(128, 1)
    # cin halves: E_q = 0:128, E_s = 128:256 (identity co order)
    wcc_half = {}
    for key, c0 in (('q', 0), ('s', 128)):
        wt = np.zeros((9, 128, 128), f32)
        for t in range(9):
            wt[t] = wcc_eff[:, c0:c0 + 128, t // 3, t % 3].T
        wcc_half[key] = wt

    wv = np.ascontiguousarray(g['cv_w'][:, :, 0, 0].T)
    bvrow = g['cv_b'].reshape(1, 128)
    wk1 = np.ascontiguousarray(g['k1_w'][:, :, 0, 0].T)        # [128, 64]
    wk2n = np.ascontiguousarray((-g['k2_w'][:, :, 0, 0]).T)
    wq1 = np.ascontiguousarray(g['q1_w'][:, :, 0, 0].T)
    wq2 = np.ascontiguousarray(g['q2_w'][:, :, 0, 0].T)
    bk1 = g['k1_b']; bk2n = -g['k2_b']; bq1 = g['q1_b']; bq2 = g['q2_b']
    w1t = np.ascontiguousarray(g['g1_w'].T) / float(N)
    b1 = g['g1_b'].reshape(CR, 1)
    w2t = np.ascontiguousarray(g['g2_w'].T)
    b2 = g['g2_b'].reshape(128, 1)

    def pad_img(x, h):
        out = np.zeros((256, NROW_IN, PW), f32)
        r0, r1 = 32 * h - 1, 32 * h + 33
        cr0, cr1 = max(r0, 0), min(r1, H)
        out[:, cr0 - r0:cr1 - r0, 1:65] = x[:, cr0:cr1, :]
        return out.reshape(2, 128, NROW_IN * PW)

    P = np.arange(128)
    in_maps = []
    for c in range(8):
        b, r = c // 4, c % 4
        role_s = r < 2
        h = r % 2
        img_full = g['s'][b] if role_s else g['q'][b]
        # x1 slot = query for S, key for Q; x2 = the other. "mine" = my image.
        if role_s:
            w1m, w1o, b1m, b1o = wq1, wq2, bq1, bq2      # query: [q1(s); q2(q)]
            w2m, w2o, b2m, b2o = wk1, wk2n, bk1, bk2n    # key:   [k1(s); -k2(q)]
        else:
            w1m, w1o, b1m, b1o = wk2n, wk1, bk2n, bk1    # key:   [-k2(q); k1(s)]
            w2m, w2o, b2m, b2o = wq2, wq1, bq2, bq1      # query: [q2(q); q1(s)]
        wproj = np.concatenate([w1m, w1o, w2m, w2o], axis=1)  # [128, 256]
        bx1 = np.concatenate([b1m, b1o]).reshape(128, 1)
        bx2 = np.concatenate([b2m, b2o]).reshape(128, 1)
        d = {
            'img': pad_img(img_full, h).astype(np.float16),
            'wstem': (ws_s if role_s else ws_q).astype(np.float16),
            'bstem': bs_s if role_s else bs_q,
            'wproj': wproj.astype(np.float16), 'bx1': bx1, 'bx2': bx2,
            'wv': wv.astype(np.float16), 'bvrow': bvrow,
            'w1t': w1t, 'b1': b1, 'w2t': w2t, 'b2': b2,
            'wcc': wcc_half['s' if role_s else 'q'].astype(np.float16),
        }
        # z granule masks: from sec r^2 take granule 0 (its local half = mine),
        # from sec r^3 take granule 1 (its other half = mine)
        msk = np.zeros(8, f32)
        msk[(r ^ 2) * 2 + 0] = 1.0
        msk[(r ^ 3) * 2 + 1] = 1.0
        d['zmsk'] = np.tile(msk, (128, 1))
        in_maps.append(d)
    return in_maps, bcc_eff


def _assemble(results, bcc_eff):
    cpam = np.zeros((B, C, H, W), np.float32)
    e_q = np.zeros((B, C, H, W), np.float32)
    e_s = np.zeros((B, C, H, W), np.float32)
    for c in range(8):
        b, r = c // 4, c % 4
        h = r % 2
        e_half = results[c]['o_e'].reshape(C, 32, W)
        tgt = e_s if r < 2 else e_q
        tgt[b, :, 32 * h:32 * h + 32, :] = e_half
    for b in range(2):
        for h in range(2):
            cs, cq = 4 * b + h, 4 * b + 2 + h
            part = (results[cs]['o_cc'].astype(np.float32) +
                    results[cq]['o_cc'].astype(np.float32)).reshape(C, 32, W)
            oth_s, oth_q = 4 * b + (1 - h), 4 * b + 2 + (1 - h)
            if h == 1:   # my row 0 needs the half-0 cores' last-row effect
                part[:, 0, :] += results[oth_s]['o_corr'][:, 0:64]
                part[:, 0, :] += results[oth_q]['o_corr'][:, 0:64]
            else:        # my row 31 needs the half-1 cores' first-row effect
                part[:, 31, :] += results[oth_s]['o_corr'][:, 64:128]
                part[:, 31, :] += results[oth_q]['o_corr'][:, 64:128]
            cpam[b, :, 32 * h:32 * h + 32, :] = np.maximum(
                part + bcc_eff[:, :, None], 0.0)
    return cpam, e_q, e_s


def kernel(**inputs):
    nc = build_program()
    in_maps, bcc_eff = _prep_inputs(inputs)
    res = bass_utils.run_bass_kernel_spmd(nc, in_maps, core_ids=list(range(8)))
    return _assemble(res.results, bcc_eff)


def kernel_traced(**inputs):
    nc = build_program()
    in_maps, bcc_eff = _prep_inputs(inputs)
    exec_ns = None
    try:
        res = bass_utils.run_bass_kernel_spmd(nc, in_maps, core_ids=list(range(8)),
                                              trace=True)
        exec_ns = res.exec_time_ns
    except Exception:
        res = bass_utils.run_bass_kernel_spmd(nc, in_maps, core_ids=list(range(8)))
    if exec_ns is None:
        try:
            from concourse.timeline_sim import TimelineSim
            exec_ns = int(TimelineSim(nc, no_exec=True, trace=False).simulate())
        except Exception:
            exec_ns = -1
    return _assemble(res.results, bcc_eff), exec_ns
